# revision 1
# baseline (speedup 1.0000x reference)
"""GNN message-passing (segment-product) kernel for 8 Trainium2 NeuronCores.

Computation (see problem reference):
    h = x @ W                                  [N, 64]
    prod[d] = product of h[src[e]] over incoming edges e of d (1 if none)
    neigh = where(deg > 0, prod, 0)
    out = neigh @ V.T                          [N, 256]

Distribution (1D dst-partition):
  - Nodes are sorted by in-degree and dealt round-robin to the 8 cores so all
    cores share one SPMD program (identical tile/padding schedule).
  - Each core computes h for its shard (fp16 x/W on the PE, fp32 PSUM) and an
    AllGather replicates the full fp32 h table to every core's HBM.
  - Edge gathering uses dma_gather (one descriptor per edge row, thousands
    per instruction).  Its int16 indices only span 32k rows, so the 100k-row
    table is processed as 4 chunks of 25088 rows; per (tile-group, chunk) the
    incoming edges are padded to a common per-chunk count K with indices
    pointing at a chunk-local ones-row (zeros-row for isolated nodes).
  - Each chunk band is fold-multiplied (fp32 on VectorE) to a partial
    product; the partials multiply into neigh.  fp16 folds would overflow
    (partial products exceed 65504), so everything after the gather is fp32.
  - PE transposes neigh tiles and applies V^T; per-core results DMA out and
    the host inverse-permutes the shards into the full [N, 256] output.
"""

import math
import numpy as np
from contextlib import ExitStack

import concourse.bass as bass
import concourse.bacc as bacc
import concourse.mybir as mybir
import concourse.tile as tile
from concourse import bass_utils
from concourse.masks import make_identity

P = 128
NCORES = 8
NCHUNK = 4
G = 4            # tiles per gather group


def _host_prep(x, W, V, src, dst):
    N, F = x.shape
    R = W.shape[1]
    H = V.shape[0]
    assert N % NCORES == 0
    npc = N // NCORES
    T = (npc + 4 + P - 1) // P
    SHARD = T * P
    CH = (NCORES * P * T) // NCHUNK

    deg = np.bincount(dst, minlength=N)
    order = np.argsort(deg, kind="stable")
    perm = [order[c::NCORES] for c in range(NCORES)]

    # gather-id of node at (core c, slot i): (c*P + i%P)*T + i//P
    pos_g = np.empty(N, dtype=np.int64)
    for c in range(NCORES):
        i = np.arange(npc)
        pos_g[perm[c]] = (c * P + (i % P)) * T + i // P

    # per-chunk filler rows: pad slots of core 2*ci land inside chunk ci
    i1, i0 = npc + 2, npc
    ones_loc = [((2 * ci * P + (i1 % P)) * T + i1 // P) - ci * CH
                for ci in range(NCHUNK)]
    zero_loc = [((2 * ci * P + (i0 % P)) * T + i0 // P) - ci * CH
                for ci in range(NCHUNK)]
    for ci in range(NCHUNK):
        assert 0 <= ones_loc[ci] < CH <= 32768 and 0 <= zero_loc[ci] < CH
    ones_p, ones_t = i1 % P, i1 // P

    # CSR of incoming edges by dst
    edge_order = np.argsort(dst, kind="stable")
    src_sorted = src[edge_order]
    starts = np.zeros(N + 1, dtype=np.int64)
    np.cumsum(np.bincount(dst, minlength=N), out=starts[1:])

    ngroups = math.ceil(T / G)
    nbr = []  # nbr[c][i] = list of NCHUNK arrays of chunk-local ids
    Ktab = np.zeros((ngroups, NCHUNK), dtype=np.int64)
    for c in range(NCORES):
        per_core = []
        for i in range(npc):
            n = perm[c][i]
            g_ids = pos_g[src_sorted[starts[n]:starts[n + 1]]]
            ci = g_ids // CH
            bucket = [g_ids[ci == k] - k * CH for k in range(NCHUNK)]
            per_core.append(bucket)
            grp = i // (P * G)
            for k in range(NCHUNK):
                if len(bucket[k]) > Ktab[grp, k]:
                    Ktab[grp, k] = len(bucket[k])
        nbr.append(per_core)

    # wrapped-int16 index planes, one [P, 8*G_eff*K] block per (group, chunk)
    gathers = []  # (grp, t0, g_eff, ci, K, col_off, num_idxs)
    col = 0
    for grp in range(ngroups):
        t0 = grp * G
        g_eff = min(G, T - t0)
        for ci in range(NCHUNK):
            K = int(Ktab[grp, ci])
            if K == 0:
                continue
            n_idx = P * g_eff * K
            gathers.append((grp, t0, g_eff, ci, K, col, n_idx))
            col += n_idx // 16
    TOTW = col

    idx_arrs = []
    for c in range(NCORES):
        plane = np.zeros((P, TOTW), dtype=np.uint16)
        for (grp, t0, g_eff, ci, K, coff, n_idx) in gathers:
            unw = np.full(n_idx, ones_loc[ci], dtype=np.uint16)
            for gj in range(g_eff):
                t = t0 + gj
                for p in range(P):
                    i = t * P + p
                    base = (gj * K) * P + p  # slot = j*128+p, j = gj*K+k
                    if i >= npc or deg[perm[c][i]] == 0:
                        for k in range(K):
                            unw[base + k * P] = zero_loc[ci]
                        continue
                    bucket = nbr[c][i][ci]
                    for k in range(len(bucket)):
                        unw[base + k * P] = bucket[k]
            w = unw.reshape(n_idx // 16, 16).T  # [16, n/16]
            plane[:, coff:coff + n_idx // 16] = np.tile(w, (8, 1))
        idx_arrs.append(plane.view(np.int16))

    xt_arrs = []
    for c in range(NCORES):
        xs = np.zeros((F, SHARD), dtype=np.float16)
        xs[:, :npc] = x[perm[c]].astype(np.float16).T
        xt_arrs.append(np.ascontiguousarray(xs))

    KB = F // P
    w_re = np.zeros((P, KB * R), dtype=np.float16)
    Wf = W.astype(np.float16)
    for cb in range(KB):
        w_re[:, cb * R:(cb + 1) * R] = Wf[cb * P:(cb + 1) * P, :]
    v_t = np.ascontiguousarray(V.T.astype(np.float32))  # [R, H]

    meta = dict(
        N=N, F=F, R=R, H=H, npc=npc, T=T, SHARD=SHARD, KB=KB, CH=CH,
        gathers=gathers, TOTW=TOTW, ngroups=ngroups,
        ones_p=ones_p, ones_t=ones_t,
        GMAXW=int(max(sum(K for (g2, _, _, _, K, _, _) in gathers if g2 == g)
                      * min(G, T - g * G) for g in range(ngroups))),
        IXW=int(max(n // 16 for (_, _, _, _, _, _, n) in gathers)),
    )
    return meta, perm, idx_arrs, xt_arrs, w_re, v_t


def _build_program(meta):
    T = meta["T"]
    SHARD = meta["SHARD"]
    F = meta["F"]
    R = meta["R"]
    H = meta["H"]
    KB = meta["KB"]
    TOTW = meta["TOTW"]
    gathers = meta["gathers"]
    ngroups = meta["ngroups"]
    CH = meta["CH"]
    f16 = mybir.dt.float16
    f32 = mybir.dt.float32

    nc = bacc.Bacc(
        "TRN2", target_bir_lowering=False, debug=False,
        enable_asserts=False, num_devices=NCORES,
    )
    x_t = nc.dram_tensor("x_t", [F, SHARD], f16, kind="ExternalInput")
    w_re = nc.dram_tensor("w_re", [P, KB * R], f16, kind="ExternalInput")
    v_t = nc.dram_tensor("v_t", [R, H], f32, kind="ExternalInput")
    idx = nc.dram_tensor("idx", [P, TOTW], mybir.dt.int16, kind="ExternalInput")
    out = nc.dram_tensor("out", [SHARD, H], f32, kind="ExternalOutput")

    by_grp = [[] for _ in range(ngroups)]
    for ga in gathers:
        by_grp[ga[0]].append(ga)

    with tile.TileContext(nc) as tc:
        with ExitStack() as ctx:
            dram = ctx.enter_context(tc.tile_pool(name="dram", bufs=1, space="DRAM"))
            sb = ctx.enter_context(tc.tile_pool(name="sb", bufs=1))
            ps1 = ctx.enter_context(tc.tile_pool(name="ps1", bufs=2, space="PSUM"))
            ps_tr = ctx.enter_context(tc.tile_pool(name="ps_tr", bufs=2, space="PSUM"))
            ps_out = ctx.enter_context(tc.tile_pool(name="ps_out", bufs=2, space="PSUM"))
            xt_pool = ctx.enter_context(tc.tile_pool(name="xt_pool", bufs=3))
            ht_pool = ctx.enter_context(tc.tile_pool(name="ht_pool", bufs=3))
            ix_pool = ctx.enter_context(tc.tile_pool(name="ix_pool", bufs=3))
            g_pool = ctx.enter_context(tc.tile_pool(name="g_pool", bufs=2))
            nb_pool = ctx.enter_context(tc.tile_pool(name="nb_pool", bufs=2))
            nt_pool = ctx.enter_context(tc.tile_pool(name="nt_pool", bufs=3))
            o_pool = ctx.enter_context(tc.tile_pool(name="o_pool", bufs=3))

            h_shard = dram.tile([P, T * R], f32)
            h_full = dram.tile([NCORES * P, T * R], f32, addr_space="Shared")

            v_sb = sb.tile([R, H], f32)
            nc.sync.dma_start(out=v_sb[:], in_=v_t[:, :])
            w_sb = sb.tile([P, KB * R], f16)
            nc.sync.dma_start(out=w_sb[:], in_=w_re[:, :])
            ident = sb.tile([P, P], f32)
            make_identity(nc, ident[:])
            h_stage = sb.tile([P, T * R], f32)

            # ---- phase 1: h = x @ W (per-shard), fp32 ----
            x_view = x_t[:, :].rearrange("(c p) n -> p c n", p=P)
            BLK = 512
            nblk = math.ceil(SHARD / BLK)
            for b in range(nblk):
                c0 = b * BLK
                cols = min(BLK, SHARD - c0)
                xt_b = xt_pool.tile([P, KB, BLK], f16, tag="xt")
                nc.sync.dma_start(
                    out=xt_b[:, :, :cols], in_=x_view[:, :, c0:c0 + cols]
                )
                h_psum = ps1.tile([R, BLK], f32, tag="h_psum")
                for cb in range(KB):
                    nc.tensor.matmul(
                        out=h_psum[:, :cols],
                        lhsT=w_sb[:, cb * R:(cb + 1) * R],
                        rhs=xt_b[:, cb, :cols],
                        start=(cb == 0),
                        stop=(cb == KB - 1),
                    )
                ht_b = ht_pool.tile([R, BLK], f32, tag="ht")
                nc.scalar.copy(out=ht_b[:, :cols], in_=h_psum[:, :cols])
                for j in range(cols // P):
                    t = (c0 // P) + j
                    tr1 = ps_tr.tile([P, R], f32, tag="tr")
                    nc.tensor.transpose(
                        out=tr1[:],
                        in_=ht_b[:, j * P:(j + 1) * P],
                        identity=ident[:R, :R],
                    )
                    nc.scalar.copy(
                        out=h_stage[:, t * R:(t + 1) * R], in_=tr1[:]
                    )
            nc.sync.dma_start(out=h_shard[:], in_=h_stage[:])
            # ones row for product padding (node slot npc+2 of every core)
            ones_sb = sb.tile([1, R], f32)
            nc.vector.memset(ones_sb[:], 1.0)
            op_, ot_ = meta["ones_p"], meta["ones_t"]
            nc.sync.dma_start(
                out=h_shard[op_:op_ + 1, ot_ * R:(ot_ + 1) * R], in_=ones_sb[:]
            )
            nc.gpsimd.collective_compute(
                "AllGather",
                mybir.AluOpType.bypass,
                replica_groups=[list(range(NCORES))],
                ins=[h_shard[:].opt()],
                outs=[h_full[:].opt()],
            )
            h_rows = h_full[:, :].rearrange("q (t m) -> (q t) m", m=R)

            # ---- phase 2: chunked gathers, per-band folds, combine, @ V^T ----
            GMAXW = meta["GMAXW"]
            IXW = meta["IXW"]
            for grp in range(ngroups):
                glist = by_grp[grp]
                if not glist:
                    continue
                g_eff = glist[0][2]
                t0 = glist[0][1]
                g_sb = g_pool.tile([P, GMAXW * R], f32, tag="g")
                offs = []
                o = 0
                for (_, _, _, ci, K, coff, n_idx) in glist:
                    ix = ix_pool.tile([P, IXW], mybir.dt.int16, tag="ix")
                    wcols = n_idx // 16
                    nc.sync.dma_start(
                        out=ix[:, :wcols], in_=idx[:, coff:coff + wcols]
                    )
                    band = g_sb[:, o * R:(o + g_eff * K) * R]
                    nc.gpsimd.dma_gather(
                        out_ap=band.rearrange("p (a b) -> p a b", b=R),
                        in_ap=h_rows[ci * CH:(ci + 1) * CH, :],
                        idxs_ap=ix[:, :wcols],
                        num_idxs=n_idx,
                        num_idxs_reg=n_idx,
                        elem_size=R,
                        single_packet=False,
                    )
                    offs.append((o, K))
                    o += g_eff * K
                # fold each band down to its first R-column block
                for (bo, K) in offs:
                    b3 = g_sb[:, bo * R:(bo + g_eff * K) * R].rearrange(
                        "p (g w) -> p g w", g=g_eff
                    )
                    m = K
                    while m > 1:
                        if m % 2:
                            nc.vector.tensor_mul(
                                out=b3[:, :, 0:R],
                                in0=b3[:, :, 0:R],
                                in1=b3[:, :, (m - 1) * R:m * R],
                            )
                            m -= 1
                            if m == 1:
                                break
                        half = m // 2
                        nc.vector.tensor_mul(
                            out=b3[:, :, :half * R],
                            in0=b3[:, :, :half * R],
                            in1=b3[:, :, half * R:m * R],
                        )
                        m = half
                # combine partial products into neigh tile
                nb = nb_pool.tile([P, G, R], f32, tag="nb")

                def band3(off_k):
                    bo, K = off_k
                    return g_sb[:, bo * R:(bo + g_eff * K) * R].rearrange(
                        "p (g w) -> p g w", g=g_eff
                    )

                if len(offs) == 1:
                    nc.vector.tensor_copy(
                        out=nb[:, :g_eff, :], in_=band3(offs[0])[:, :, 0:R]
                    )
                else:
                    nc.vector.tensor_mul(
                        out=nb[:, :g_eff, :],
                        in0=band3(offs[0])[:, :, 0:R],
                        in1=band3(offs[1])[:, :, 0:R],
                    )
                    for off_k in offs[2:]:
                        nc.vector.tensor_mul(
                            out=nb[:, :g_eff, :],
                            in0=nb[:, :g_eff, :],
                            in1=band3(off_k)[:, :, 0:R],
                        )
                for gj in range(g_eff):
                    t = t0 + gj
                    tr2 = ps_tr.tile([R, P], f32, tag="tr")
                    nc.tensor.transpose(
                        out=tr2[:], in_=nb[:, gj, :], identity=ident[:]
                    )
                    nt = nt_pool.tile([R, P], f32, tag="nt")
                    nc.scalar.copy(out=nt[:], in_=tr2[:])
                    o_psum = ps_out.tile([P, H], f32, tag="o_psum")
                    nc.tensor.matmul(
                        out=o_psum[:], lhsT=nt[:], rhs=v_sb[:],
                        start=True, stop=True,
                    )
                    o_sb = o_pool.tile([P, H], f32, tag="o_sb")
                    nc.scalar.copy(out=o_sb[:], in_=o_psum[:])
                    nc.sync.dma_start(
                        out=out[t * P:(t + 1) * P, :], in_=o_sb[:]
                    )
    nc.compile()
    return nc


def kernel(x, W, V, src, dst):
    x = np.asarray(x)
    W = np.asarray(W)
    V = np.asarray(V)
    src = np.asarray(src)
    dst = np.asarray(dst)
    meta, perm, idx_arrs, xt_arrs, w_re, v_t = _host_prep(x, W, V, src, dst)
    nc = _build_program(meta)
    in_maps = [
        {"x_t": xt_arrs[c], "w_re": w_re, "v_t": v_t, "idx": idx_arrs[c]}
        for c in range(NCORES)
    ]
    res = bass_utils.run_bass_kernel_spmd(nc, in_maps, core_ids=list(range(NCORES)))
    out_full = np.empty((meta["N"], meta["H"]), dtype=np.float32)
    for c in range(NCORES):
        out_full[perm[c]] = res.results[c]["out"][:meta["npc"]]
    return out_full

